# revision 23
# baseline (speedup 1.0000x reference)
"""DilatedAttention Trainium2 kernel (shared-compute restructure).

B=2, n=16 heads, T=8192, d=64. Three dilated passes (S,r) in
[(512,1),(1024,2),(2048,4)]; head h uses segments (h%r)+r*j; causal
softmax inside each segment; out = (p1+p2+p3)/3.

The passes NEST: for a 512-query-block, pass-2's causal scores over its
1024-segment contain pass-1's diagonal block, and pass-3's 2048-segment
contains both. Accumulating the AV numerator (plus a ones-row
denominator) nearest-keys-first per query block and snapshotting PSUM at
nested prefix boundaries yields every pass's (numerator, denominator)
without recomputing shared blocks: 38912 score columns per (b,h) pair vs
56320 for independent passes. Normalization and the pass-weighted
combine happen on the host (off the device critical path).

Sharding: 32 (b,h) pairs -> 8 cores x 4 pairs. Host permutes each
head's 16 512-blocks so selected segments are slot-contiguous:
  slots 0-3   : the head's pass-3 2048-segment (4 blocks, orig order)
  slots 4-9   : the 3 pass-2 1024-segments not inside pass-3
  slots 10-15 : the 6 remaining blocks (pass-1 only)
Device writes per-snapshot [65, 512] fp16 tiles (64 numerator rows over
V/3 + denominator row); host divides, applies the pass-sharing weight
(1..3), and scatters back into (B, n, T, d).
"""

import sys
import os

for _p in ("/opt/trn_rl_repo", "/root/.axon_site/_ro/trn_rl_repo"):
    if os.path.isdir(_p) and _p not in sys.path:
        sys.path.insert(0, _p)

import numpy as np
import ml_dtypes  # noqa: F401

import concourse.bass as bass
import concourse.tile as tile
from concourse import mybir
from concourse.bass_utils import run_bass_kernel_spmd

# ---------------------------------------------------------------- constants
B, NH, T, D = 2, 16, 8192, 64
BLK = 512
NBLK = T // BLK            # 16
TS = NBLK * BLK            # 8192 tokens per pair (no duplication)
NCH = TS // 128            # 64 key chunks
N_CORES = 8
PAIRS_PER_CORE = 4
NSNAP = 23                 # snapshot slots per pair (parity 0 uses 22)

F32 = mybir.dt.float32
BF16 = mybir.dt.bfloat16
FP16 = mybir.dt.float16


# ----------------------------------------------------------- shared schedule
def _slot_map(h):
    a, p = h % 4, h % 2
    p3 = [4 * a + i for i in range(4)]
    s_in = 2 * a + p
    p2segs = [s for s in (p, p + 2, p + 4, p + 6) if s != s_in]
    p2 = [b for s in p2segs for b in (2 * s, 2 * s + 1)]
    rest = [b for b in range(16) if b not in p3 and b not in p2]
    return p3 + p2 + rest


def _pair_schedule(p):
    """Group schedule for head parity p.
    Returns (slots, snaps): slots[s] = [(subs, snap_weight_or_None), ...]
    where subs = [(kc, off, nq, qo, diag_off), ...]; snaps = [(slot, w), ...]
    in device snapshot-emission order. Wedge subchunks pack [0:1280) with
    no psum-bank crossing (sub3's 128 cols slot in at 896)."""
    def wedge(s):
        k = 4 * s
        return [(k, 0, 512, 0, 0), (k + 1, 512, 384, 128, 512),
                (k + 3, 896, 128, 384, 896), (k + 2, 1024, 256, 256, 1024)]

    table = {}
    table[0] = (3 if p == 0 else 2, [], {})
    table[1] = (1, [0], {4: 2 if p == 0 else 1})
    table[2] = ((1 if p == 0 else 2), [1, 0], {8: 1})
    table[3] = (1, [2, 1, 0], ({12: 1} if p == 0 else {4: 1, 12: 1}))
    for s in (4, 6, 8):
        table[s] = (2, [], {})
        table[s + 1] = (1, [s], {4: 1})
    for s in range(10, 16):
        table[s] = (1, [], {})

    slots, snaps = [], []
    for s in range(16):
        w_wedge, obs, marks = table[s]
        groups = [(wedge(s), w_wedge)]
        snaps.append((s, w_wedge))
        stream = [4 * j + t for j in obs for t in range(4)]
        prev = 0
        for cut in sorted(marks):
            seg = stream[prev:cut]
            packs = [seg[i:i + 3] for i in range(0, len(seg), 3)]
            for gi, pk in enumerate(packs):
                w = marks[cut] if gi == len(packs) - 1 else None
                groups.append((
                    [(kc, 512 * i, 512, 0, None) for i, kc in enumerate(pk)],
                    w))
                if w is not None:
                    snaps.append((s, w))
            prev = cut
        slots.append(groups)
    return slots, snaps


# ------------------------------------------------------------- tile patch
def _patched_drain_and_barrier(self, tick_clock, wait_clock):
    # This walrus build rejects a CTRL Drain carrying >1 sync wait; split the
    # kernel-tail waits across one drain each.
    nc = self.nc
    di = nc.sync.drain()
    wait_clock.add_sem_waits(di.ins, tile.ScopedClock({None: tick_clock.global_clock}))
    si = di.ins.sync_info
    waits = list(si.on_wait)
    si.on_wait = waits[:1]
    proto = type(si)
    for w in waits[1:]:
        d2 = nc.sync.drain()
        d2.ins.sync_info = proto(on_wait=[w], on_update=[])
    nc.all_engine_barrier()
    popped = nc._tile_sem_poison_stack.pop()
    assert popped is self._sem_poison
    nc.clear_and_free_semaphores(list(self.sems.allocated().values()))
    nc.all_engine_barrier()


tile.TileContext._drain_and_barrier = _patched_drain_and_barrier


def _split_excess_waits(nc, max_waits=1):
    """This walrus build allows at most 2 sync waits per engine instruction
    (1 for CTRL/Drain). Move excess waits onto same-engine NOPs inserted
    immediately before the offending instruction."""
    proto = None
    for bbw in nc.bb_map.values():
        il = bbw.bb.instructions  # live list
        i = 0
        while i < len(il):
            inst = il[i]
            si = inst.sync_info
            limit = 1 if type(inst).__name__ == "InstDrain" else max_waits
            if si is not None and len(si.on_wait) > limit:
                waits = list(si.on_wait)
                if proto is None:
                    proto = type(si)
                keep = waits[len(waits) - limit:]
                over = waits[:len(waits) - limit]
                si.on_wait = keep
                # chunk the overflow onto nops (each nop takes max_waits)
                chunks = [over[j:j + max_waits]
                          for j in range(0, len(over), max_waits)]
                for ci, ch in enumerate(chunks):
                    bi = nc.engines[inst.engine].nop(nofuse=True)
                    nop_inst = bi.ins
                    # nop() appended nop_inst to the current bb; relocate it
                    for bb2 in nc.bb_map.values():
                        il2 = bb2.bb.instructions
                        if il2 and il2[-1] is nop_inst:
                            il2.pop()
                            break
                    nop_inst.sync_info = proto(on_wait=ch, on_update=[])
                    il.insert(i + ci, nop_inst)
                i += len(chunks)
            i += 1


# ------------------------------------------------------------ device program
_SIM_SAFE = [False]

# Schraudolph exp on DVE: i32(A8*score + B) bit-read as f32 ~= exp(score/8).
# C=486411 centers the error (zero mean, ~1.8% rms); numerator/denominator
# share the same values so the softmax ratio cancels most of it (end-to-end
# ~3e-3). Only off-diagonal groups are eligible (no causal mask interplay).
_SEXP_A8 = float((1 << 23) / np.log(2) / 8.0)
_SEXP_B = float((127 << 23) - 486411)


def build_program(n_pairs=PAIRS_PER_CORE):
    n_cpl = (n_pairs + 1) // 2
    nc = bass.Bass()
    qt_in = nc.declare_dram_parameter("QT", [n_cpl, 128, TS], BF16,
                                      isOutput=False)
    kt_in = nc.declare_dram_parameter("KT", [n_pairs, 128, TS], BF16,
                                      isOutput=False)
    v1_in = nc.declare_dram_parameter("V1", [n_pairs, 128, 66 * NCH], BF16,
                                      isOutput=False)
    ob_out = nc.declare_dram_parameter("Ob", [n_pairs, 65, NSNAP, 512], FP16,
                                       isOutput=True)

    couples = [
        [p for p in (2 * c, 2 * c + 1) if p < n_pairs] for c in range(n_cpl)
    ]

    with tile.TileContext(nc) as tc:
        with (
            tc.tile_pool(name="qt", bufs=8) as qt_p,
            tc.tile_pool(name="kt", bufs=16) as kt_p,
            tc.tile_pool(name="v1", bufs=16) as v1_p,
            tc.tile_pool(name="ex", bufs=5) as ex_p,
            tc.tile_pool(name="sx", bufs=3) as sx_p,
            tc.tile_pool(name="otb", bufs=3) as otb_p,
            tc.tile_pool(name="sc", bufs=2, space="PSUM") as sc_p,
            tc.tile_pool(name="po", bufs=2, space="PSUM") as po_p,
        ):
            qt_h, kt_h, v1_h = {}, {}, {}
            QTOK = TS // 4          # tokens per load quarter
            QKC = NCH // 4          # key chunks per quarter

            def prep_alloc(c):
                """Allocate a couple's quarter tiles and return deferred
                dma_start closures in need order. Issue is dripped one per
                group so early loads get full HBM bandwidth instead of
                sharing it with every later load."""
                loads = []

                def add(pool, dct, key, width, src):
                    t = pool.tile([128, width], BF16, tag=pool.name,
                                  name=pool.name)
                    dct[key] = t
                    loads.append(lambda t=t, src=src:
                                 nc.sync.dma_start(out=t[:, :], in_=src))

                p0 = couples[c][0]
                for qi in range(4):
                    add(kt_p, kt_h, (p0, qi), QTOK,
                        kt_in[p0, :, QTOK * qi:QTOK * (qi + 1)])
                    add(qt_p, qt_h, (c, qi), QTOK,
                        qt_in[c, :, QTOK * qi:QTOK * (qi + 1)])
                    add(v1_p, v1_h, (p0, qi), 66 * QKC,
                        v1_in[p0, :, 66 * QKC * qi:66 * QKC * (qi + 1)])
                for pair in couples[c][1:]:
                    for qi in range(4):
                        add(kt_p, kt_h, (pair, qi), QTOK,
                            kt_in[pair, :, QTOK * qi:QTOK * (qi + 1)])
                        add(v1_p, v1_h, (pair, qi), 66 * QKC,
                            v1_in[pair, :, 66 * QKC * qi:66 * QKC * (qi + 1)])
                return loads

            def build_pair_closures(pair):
                """(front, back) emitters for every group of one pair.
                front = QK^T -> exp -> causal mask; back = AV accumulate
                into the block's po psum (+ snapshot copy / output DMA)."""
                slots, _ = _pair_schedule(pair % 2)
                cpl = pair // 2
                sexp_num = int(os.environ.get("SEXP_NUM", "2"))
                sexp_den = int(os.environ.get("SEXP_DEN", "3"))
                copy_eng = (nc.gpsimd if os.environ.get("COPY_ENG") == "pool"
                            else nc.vector)
                odg = [0]  # running index of off-diagonal groups
                out = []
                snap_base = 0
                for s, groups in enumerate(slots):
                    q0 = 512 * s
                    nsnap_slot = sum(1 for _, w in groups if w is not None)
                    total_cc = sum(len(g) for g, _ in groups)
                    po_t = po_p.tile([65, 512], F32, tag="po", name="po")
                    otb_t = otb_p.tile([65, 512 * nsnap_slot], FP16,
                                       tag="otb", name="otb")
                    state = {"cc": 0, "si": 0}
                    base_idx = snap_base
                    snap_base += nsnap_slot

                    def mk(subs, has_snap, is_last, po_t=po_t, otb_t=otb_t,
                           state=state, q0=q0, total_cc=total_cc,
                           base_idx=base_idx, nsnap_slot=nsnap_slot):
                        sc_t = sc_p.tile([128, 1536], F32, tag="sc", name="sc")
                        ex_t = ex_p.tile([128, 1536], BF16, tag="ex", name="ex")
                        offdiag = all(do is None for *_x, do in subs)
                        dve_exp = False
                        if offdiag:
                            dve_exp = (odg[0] % sexp_den) < sexp_num
                            odg[0] += 1
                        # contiguous spans (the wedge has a pad gap [896:1024])
                        spans = []
                        for off, end in sorted(
                                (off, off + nq) for _, off, nq, _, _ in subs):
                            if spans and off <= spans[-1][1]:
                                spans[-1][1] = max(spans[-1][1], end)
                            else:
                                spans.append([off, end])

                        def front():
                            for kc, off, nq, qo, _ in subs:
                                kt_t = kt_h[(pair, kc // QKC)]
                                klo = 128 * (kc % QKC)
                                qt_t = qt_h[(cpl, q0 // QTOK)]
                                qlo = q0 % QTOK
                                nc.tensor.matmul(
                                    sc_t[:, off:off + nq],
                                    lhsT=kt_t[:, klo:klo + 128],
                                    rhs=qt_t[:, qlo + qo:qlo + 512],
                                    start=True, stop=True,
                                )
                            if dve_exp:
                                a, b_ = spans[0][0], spans[-1][1]
                                sx_t = sx_p.tile([128, 1536], mybir.dt.int32,
                                                 tag="sx", name="sx")
                                nc.vector.tensor_scalar(
                                    out=sx_t[:, a:b_], in0=sc_t[:, a:b_],
                                    scalar1=_SEXP_A8, scalar2=_SEXP_B,
                                    op0=mybir.AluOpType.mult,
                                    op1=mybir.AluOpType.add,
                                )
                                # bit-read as f32 and narrow to bf16 on the
                                # Pool engine (SBUF->SBUF) to keep the DVE
                                # queue short ahead of the snapshot copies
                                nc.gpsimd.tensor_copy(
                                    ex_t[:, a:b_],
                                    sx_t[:, a:b_].bitcast(F32),
                                )
                            elif len(spans) == 1 or not _SIM_SAFE[0]:
                                # pad gap holds stale psum; exp of it is
                                # finite and unread
                                nc.scalar.activation(
                                    ex_t[:, spans[0][0]:spans[-1][1]],
                                    sc_t[:, spans[0][0]:spans[-1][1]],
                                    mybir.ActivationFunctionType.Exp,
                                    scale=0.125,
                                )
                            else:
                                for a, b_ in spans:
                                    nc.scalar.activation(
                                        ex_t[:, a:b_], sc_t[:, a:b_],
                                        mybir.ActivationFunctionType.Exp,
                                        scale=0.125,
                                    )
                            for _, off, nq, qo, do in subs:
                                if do is None:
                                    continue
                                # keep exp where q_local - k_local >= 0
                                nc.gpsimd.affine_select(
                                    out=ex_t[:, do:do + 128],
                                    in_=ex_t[:, do:do + 128],
                                    compare_op=mybir.AluOpType.is_ge,
                                    fill=0.0, base=0,
                                    pattern=[[1, 128]], channel_multiplier=-1,
                                )

                        def back():
                            for kc, off, nq, qo, _ in subs:
                                v1_t = v1_h[(pair, kc // QKC)]
                                vlo = 66 * (kc % QKC)
                                nc.tensor.matmul(
                                    po_t[:, qo:512],
                                    lhsT=v1_t[:, vlo:vlo + 65],
                                    rhs=ex_t[:, off:off + nq],
                                    start=(state["cc"] == 0),
                                    stop=(state["cc"] == total_cc - 1),
                                )
                                state["cc"] += 1
                            if has_snap:
                                si = state["si"]
                                state["si"] += 1
                                copy_eng.tensor_copy(
                                    otb_t[:, 512 * si:512 * (si + 1)],
                                    po_t[:, :])
                            if is_last:
                                # SBUF side must stay partition-first; the
                                # snapshot stride lives on the DRAM side
                                nc.sync.dma_start(
                                    out=ob_out[
                                        pair, :,
                                        base_idx:base_idx + nsnap_slot],
                                    in_=otb_t.rearrange("p (s c) -> p s c",
                                                        c=512),
                                )

                        return front, back

                    for gi, (subs, snap_w) in enumerate(groups):
                        out.append(mk(subs, snap_w is not None,
                                      gi == len(groups) - 1))
                return out

            # software pipeline: QK/exp of groups i+1..i+2 issue before the
            # AV of group i so the PE never head-of-line blocks on the
            # exp->mask latency or the snapshot copy WAR.
            pend = []
            depth = int(os.environ.get("KDEPTH", "2"))

            def pump(fb):
                front, back = fb
                front()
                pend.append(back)
                if len(pend) > depth:
                    pend.pop(0)()

            from collections import deque
            drip = deque()
            for c in range(len(couples)):
                drip.extend(prep_alloc(c))
            # first block's tiles up front; the rest stream behind compute
            for _ in range(3):
                drip.popleft()()
            for c, members in enumerate(couples):
                for pi, pair in enumerate(members):
                    for fb in build_pair_closures(pair):
                        pump(fb)
                        if drip:
                            drip.popleft()()
            while pend:
                pend.pop(0)()
    _split_excess_waits(nc)
    return nc


# ------------------------------------------------------------- host wrapper
_PROGRAM = None


def _get_program():
    global _PROGRAM
    if _PROGRAM is None:
        _PROGRAM = build_program()
    return _PROGRAM


_BF = ml_dtypes.bfloat16


def _marshal(qs, ks, vs):
    """[n_pairs, TS, 64] f32 triplet -> device input dict. Couple-packed
    bf16 Q^T (two pairs' d dims stacked on partitions), per-pair K^T with
    the other partition half zeroed (K=128 matmuls keep the PE warm), and
    the per-chunk strided [V/3 | 1] layout. Pure layout marshalling."""
    n_pairs = qs.shape[0]
    n_cpl = (n_pairs + 1) // 2
    QT = np.zeros((n_cpl, 128, TS), dtype=_BF)
    KT = np.zeros((n_pairs, 128, TS), dtype=_BF)
    for pair in range(n_pairs):
        lo = 64 * (pair % 2)
        QT[pair // 2, lo:lo + 64, :] = qs[pair].T.astype(_BF)
        KT[pair, lo:lo + 64, :] = ks[pair].T.astype(_BF)
    v = (vs.astype(np.float32) / 3.0).reshape(n_pairs, NCH, 128, 64)
    V1 = np.ones((n_pairs, 128, NCH, 66), dtype=_BF)
    V1[..., :64] = v.transpose(0, 2, 1, 3).astype(_BF)
    return {
        "QT": QT,
        "KT": KT,
        "V1": np.ascontiguousarray(V1.reshape(n_pairs, 128, NCH * 66)),
    }


def _shard_inputs(Q, K, V):
    """-> list of 8 dicts with permuted, marshalled per-core arrays."""
    in_maps = []
    for core in range(N_CORES):
        qs, ks, vs = [], [], []
        for pi in range(PAIRS_PER_CORE):
            flat = core * PAIRS_PER_CORE + pi
            b, h = flat // NH, flat % NH
            sm = _slot_map(h)
            for lst, src in ((qs, Q), (ks, K), (vs, V)):
                lst.append(
                    src[b, h].reshape(NBLK, BLK, D)[sm].reshape(TS, D)
                )
        in_maps.append(_marshal(np.stack(qs), np.stack(ks), np.stack(vs)))
    return in_maps


def _combine_outputs(results):
    out = np.zeros((B, NH, T, D), np.float32)
    for core in range(N_CORES):
        ob = results[core]["Ob"].astype(np.float32)  # [4, 65, NSNAP, 512]
        for pi in range(PAIRS_PER_CORE):
            flat = core * PAIRS_PER_CORE + pi
            b, h = flat // NH, flat % NH
            sm = _slot_map(h)
            _, snaps = _pair_schedule(pi % 2)
            blocks = np.zeros((NBLK, BLK, D), np.float32)
            for si, (slot, w) in enumerate(snaps):
                n = ob[pi, 0:64, si, :]          # (64, 512)
                den = ob[pi, 64, si, :]          # (512,)
                blocks[sm[slot]] += w * (n / den).T
            out[b, h] = blocks.reshape(T, D)
    return out


def kernel(Q, K, V):
    Q = np.asarray(Q, dtype=np.float32)
    K = np.asarray(K, dtype=np.float32)
    V = np.asarray(V, dtype=np.float32)
    nc = _get_program()
    in_maps = _shard_inputs(Q, K, V)
    res = run_bass_kernel_spmd(nc, in_maps, list(range(N_CORES)))
    return _combine_outputs(res.results)


if __name__ == "__main__":
    rng = np.random.default_rng(0)
    Q = rng.standard_normal((B, NH, T, D), dtype=np.float32)
    K = rng.standard_normal((B, NH, T, D), dtype=np.float32)
    V = rng.standard_normal((B, NH, T, D), dtype=np.float32)
    out = kernel(Q=Q, K=K, V=V)
    print("out", out.shape, out.dtype, float(np.abs(out).mean()))


# revision 26
# speedup vs baseline: 1.4410x; 1.4410x over previous
"""DilatedAttention Trainium2 kernel (shared-compute restructure).

B=2, n=16 heads, T=8192, d=64. Three dilated passes (S,r) in
[(512,1),(1024,2),(2048,4)]; head h uses segments (h%r)+r*j; causal
softmax inside each segment; out = (p1+p2+p3)/3.

The passes NEST: for a 512-query-block, pass-2's causal scores over its
1024-segment contain pass-1's diagonal block, and pass-3's 2048-segment
contains both. Accumulating the AV numerator (plus a ones-row
denominator) nearest-keys-first per query block and snapshotting PSUM at
nested prefix boundaries yields every pass's (numerator, denominator)
without recomputing shared blocks: 38912 score columns per (b,h) pair vs
56320 for independent passes. Normalization and the pass-weighted
combine happen on the host (off the device critical path).

Sharding: 32 (b,h) pairs -> 8 cores x 4 pairs. Host permutes each
head's 16 512-blocks so selected segments are slot-contiguous:
  slots 0-3   : the head's pass-3 2048-segment (4 blocks, orig order)
  slots 4-9   : the 3 pass-2 1024-segments not inside pass-3
  slots 10-15 : the 6 remaining blocks (pass-1 only)
Device writes per-snapshot [65, 512] fp16 tiles (64 numerator rows over
V/3 + denominator row); host divides, applies the pass-sharing weight
(1..3), and scatters back into (B, n, T, d).
"""

import sys
import os

for _p in ("/opt/trn_rl_repo", "/root/.axon_site/_ro/trn_rl_repo"):
    if os.path.isdir(_p) and _p not in sys.path:
        sys.path.insert(0, _p)

import numpy as np
import ml_dtypes  # noqa: F401

import concourse.bass as bass
import concourse.tile as tile
from concourse import mybir
from concourse.bass_utils import run_bass_kernel_spmd

# ---------------------------------------------------------------- constants
B, NH, T, D = 2, 16, 8192, 64
BLK = 512
NBLK = T // BLK            # 16
TS = NBLK * BLK            # 8192 tokens per pair (no duplication)
NCH = TS // 128            # 64 key chunks
N_CORES = 8
PAIRS_PER_CORE = 4
NSNAP = 23                 # snapshot slots per pair (parity 0 uses 22)

F32 = mybir.dt.float32
BF16 = mybir.dt.bfloat16
FP16 = mybir.dt.float16


# ----------------------------------------------------------- shared schedule
def _slot_map(h):
    a, p = h % 4, h % 2
    p3 = [4 * a + i for i in range(4)]
    s_in = 2 * a + p
    p2segs = [s for s in (p, p + 2, p + 4, p + 6) if s != s_in]
    p2 = [b for s in p2segs for b in (2 * s, 2 * s + 1)]
    rest = [b for b in range(16) if b not in p3 and b not in p2]
    return p3 + p2 + rest


def _pair_schedule(p):
    """Group schedule for head parity p.
    Returns (slots, snaps): slots[s] = [(subs, snap_weight_or_None), ...]
    where subs = [(kc, off, nq, qo, diag_off), ...]; snaps = [(slot, w), ...]
    in device snapshot-emission order. Wedge subchunks pack [0:1280) with
    no psum-bank crossing (sub3's 128 cols slot in at 896)."""
    def wedge(s):
        k = 4 * s
        return [(k, 0, 512, 0, 0), (k + 1, 512, 384, 128, 512),
                (k + 3, 896, 128, 384, 896), (k + 2, 1024, 256, 256, 1024)]

    table = {}
    table[0] = (3 if p == 0 else 2, [], {})
    table[1] = (1, [0], {4: 2 if p == 0 else 1})
    table[2] = ((1 if p == 0 else 2), [1, 0], {8: 1})
    table[3] = (1, [2, 1, 0], ({12: 1} if p == 0 else {4: 1, 12: 1}))
    for s in (4, 6, 8):
        table[s] = (2, [], {})
        table[s + 1] = (1, [s], {4: 1})
    for s in range(10, 16):
        table[s] = (1, [], {})

    slots, snaps = [], []
    for s in range(16):
        w_wedge, obs, marks = table[s]
        groups = [(wedge(s), w_wedge)]
        snaps.append((s, w_wedge))
        stream = [4 * j + t for j in obs for t in range(4)]
        prev = 0
        for cut in sorted(marks):
            seg = stream[prev:cut]
            packs = [seg[i:i + 3] for i in range(0, len(seg), 3)]
            for gi, pk in enumerate(packs):
                w = marks[cut] if gi == len(packs) - 1 else None
                groups.append((
                    [(kc, 512 * i, 512, 0, None) for i, kc in enumerate(pk)],
                    w))
                if w is not None:
                    snaps.append((s, w))
            prev = cut
        slots.append(groups)
    return slots, snaps


# ------------------------------------------------------------- tile patch
def _patched_drain_and_barrier(self, tick_clock, wait_clock):
    # This walrus build rejects a CTRL Drain carrying >1 sync wait; split the
    # kernel-tail waits across one drain each.
    nc = self.nc
    di = nc.sync.drain()
    wait_clock.add_sem_waits(di.ins, tile.ScopedClock({None: tick_clock.global_clock}))
    si = di.ins.sync_info
    waits = list(si.on_wait)
    si.on_wait = waits[:1]
    proto = type(si)
    for w in waits[1:]:
        d2 = nc.sync.drain()
        d2.ins.sync_info = proto(on_wait=[w], on_update=[])
    nc.all_engine_barrier()
    popped = nc._tile_sem_poison_stack.pop()
    assert popped is self._sem_poison
    nc.clear_and_free_semaphores(list(self.sems.allocated().values()))
    nc.all_engine_barrier()


tile.TileContext._drain_and_barrier = _patched_drain_and_barrier


def _split_excess_waits(nc, max_waits=1):
    """This walrus build allows at most 2 sync waits per engine instruction
    (1 for CTRL/Drain). Move excess waits onto same-engine NOPs inserted
    immediately before the offending instruction."""
    proto = None
    for bbw in nc.bb_map.values():
        il = bbw.bb.instructions  # live list
        i = 0
        while i < len(il):
            inst = il[i]
            si = inst.sync_info
            limit = 1 if type(inst).__name__ == "InstDrain" else max_waits
            if si is not None and len(si.on_wait) > limit:
                waits = list(si.on_wait)
                if proto is None:
                    proto = type(si)
                keep = waits[len(waits) - limit:]
                over = waits[:len(waits) - limit]
                si.on_wait = keep
                # chunk the overflow onto nops (each nop takes max_waits)
                chunks = [over[j:j + max_waits]
                          for j in range(0, len(over), max_waits)]
                for ci, ch in enumerate(chunks):
                    bi = nc.engines[inst.engine].nop(nofuse=True)
                    nop_inst = bi.ins
                    # nop() appended nop_inst to the current bb; relocate it
                    for bb2 in nc.bb_map.values():
                        il2 = bb2.bb.instructions
                        if il2 and il2[-1] is nop_inst:
                            il2.pop()
                            break
                    nop_inst.sync_info = proto(on_wait=ch, on_update=[])
                    il.insert(i + ci, nop_inst)
                i += len(chunks)
            i += 1


# ------------------------------------------------------------ device program
_SIM_SAFE = [False]

# Schraudolph exp on DVE: i32(A8*score + B) bit-read as f32 ~= exp(score/8).
# C=486411 centers the error (zero mean, ~1.8% rms); numerator/denominator
# share the same values so the softmax ratio cancels most of it (end-to-end
# ~3e-3). Only off-diagonal groups are eligible (no causal mask interplay).
_SEXP_A8 = float((1 << 23) / np.log(2) / 8.0)
_SEXP_B = float((127 << 23) - 486411)


def build_program(n_pairs=PAIRS_PER_CORE):
    n_cpl = (n_pairs + 1) // 2
    nc = bass.Bass()
    qt_in = nc.declare_dram_parameter("QT", [n_cpl, 128, TS], BF16,
                                      isOutput=False)
    kt_in = nc.declare_dram_parameter("KT", [n_pairs, 128, TS], BF16,
                                      isOutput=False)
    v1_in = nc.declare_dram_parameter("V1", [n_pairs, 128, 66 * NCH], BF16,
                                      isOutput=False)
    ob_out = nc.declare_dram_parameter("Ob", [n_pairs, 65, NSNAP, 512], FP16,
                                       isOutput=True)

    couples = [
        [p for p in (2 * c, 2 * c + 1) if p < n_pairs] for c in range(n_cpl)
    ]

    with tile.TileContext(nc) as tc:
        with (
            tc.tile_pool(name="qt", bufs=8) as qt_p,
            tc.tile_pool(name="kt", bufs=16) as kt_p,
            tc.tile_pool(name="v1", bufs=16) as v1_p,
            tc.tile_pool(name="ex", bufs=5) as ex_p,
            tc.tile_pool(name="sx", bufs=3) as sx_p,
            tc.tile_pool(name="otb", bufs=3) as otb_p,
            tc.tile_pool(name="sc", bufs=2, space="PSUM") as sc_p,
            tc.tile_pool(name="po", bufs=2, space="PSUM") as po_p,
        ):
            qt_h, kt_h, v1_h = {}, {}, {}
            QTOK = TS // 4          # tokens per load quarter
            QKC = NCH // 4          # key chunks per quarter

            def prep_alloc(c):
                """Allocate a couple's quarter tiles and return deferred
                dma_start closures in need order. Issue is dripped one per
                group so early loads get full HBM bandwidth instead of
                sharing it with every later load."""
                loads = []

                def add(pool, dct, key, width, src):
                    t = pool.tile([128, width], BF16, tag=pool.name,
                                  name=pool.name)
                    dct[key] = t
                    loads.append(lambda t=t, src=src:
                                 nc.sync.dma_start(out=t[:, :], in_=src))

                p0 = couples[c][0]
                for qi in range(4):
                    add(kt_p, kt_h, (p0, qi), QTOK,
                        kt_in[p0, :, QTOK * qi:QTOK * (qi + 1)])
                    add(qt_p, qt_h, (c, qi), QTOK,
                        qt_in[c, :, QTOK * qi:QTOK * (qi + 1)])
                    add(v1_p, v1_h, (p0, qi), 66 * QKC,
                        v1_in[p0, :, 66 * QKC * qi:66 * QKC * (qi + 1)])
                for pair in couples[c][1:]:
                    for qi in range(4):
                        add(kt_p, kt_h, (pair, qi), QTOK,
                            kt_in[pair, :, QTOK * qi:QTOK * (qi + 1)])
                        add(v1_p, v1_h, (pair, qi), 66 * QKC,
                            v1_in[pair, :, 66 * QKC * qi:66 * QKC * (qi + 1)])
                return loads

            def build_pair_closures(pair):
                """(front, back) emitters for every group of one pair.
                front = QK^T -> exp -> causal mask; back = AV accumulate
                into the block's po psum (+ snapshot copy / output DMA)."""
                slots, _ = _pair_schedule(pair % 2)
                cpl = pair // 2
                sexp_num = int(os.environ.get("SEXP_NUM", "2"))
                sexp_den = int(os.environ.get("SEXP_DEN", "3"))
                copy_eng = (nc.gpsimd if os.environ.get("COPY_ENG") == "pool"
                            else nc.vector)
                odg = [0]  # running index of off-diagonal groups
                out = []
                snap_base = 0
                for s, groups in enumerate(slots):
                    q0 = 512 * s
                    nsnap_slot = sum(1 for _, w in groups if w is not None)
                    total_cc = sum(len(g) for g, _ in groups)
                    po_t = po_p.tile([65, 512], F32, tag="po", name="po")
                    otb_t = otb_p.tile([65, 512 * nsnap_slot], FP16,
                                       tag="otb", name="otb")
                    state = {"cc": 0, "si": 0}
                    base_idx = snap_base
                    snap_base += nsnap_slot

                    def mk(subs, has_snap, is_last, po_t=po_t, otb_t=otb_t,
                           state=state, q0=q0, total_cc=total_cc,
                           base_idx=base_idx, nsnap_slot=nsnap_slot):
                        sc_t = sc_p.tile([128, 1536], F32, tag="sc", name="sc")
                        ex_t = ex_p.tile([128, 1536], BF16, tag="ex", name="ex")
                        offdiag = all(do is None for *_x, do in subs)
                        dve_exp = False
                        if offdiag:
                            dve_exp = (odg[0] % sexp_den) < sexp_num
                            odg[0] += 1
                        sx_t = None
                        if dve_exp:
                            sx_t = sx_p.tile([128, 1536], mybir.dt.int32,
                                             tag="sx", name="sx")
                        # contiguous spans (the wedge has a pad gap [896:1024])
                        spans = []
                        for off, end in sorted(
                                (off, off + nq) for _, off, nq, _, _ in subs):
                            if spans and off <= spans[-1][1]:
                                spans[-1][1] = max(spans[-1][1], end)
                            else:
                                spans.append([off, end])

                        def front():
                            for kc, off, nq, qo, _ in subs:
                                kt_t = kt_h[(pair, kc // QKC)]
                                klo = 128 * (kc % QKC)
                                qt_t = qt_h[(cpl, q0 // QTOK)]
                                qlo = q0 % QTOK
                                nc.tensor.matmul(
                                    sc_t[:, off:off + nq],
                                    lhsT=kt_t[:, klo:klo + 128],
                                    rhs=qt_t[:, qlo + qo:qlo + 512],
                                    start=True, stop=True,
                                )
                            if dve_exp:
                                a, b_ = spans[0][0], spans[-1][1]
                                nc.vector.tensor_scalar(
                                    out=sx_t[:, a:b_], in0=sc_t[:, a:b_],
                                    scalar1=_SEXP_A8, scalar2=_SEXP_B,
                                    op0=mybir.AluOpType.mult,
                                    op1=mybir.AluOpType.add,
                                )
                            elif len(spans) == 1 or not _SIM_SAFE[0]:
                                # pad gap holds stale psum; exp of it is
                                # finite and unread
                                nc.scalar.activation(
                                    ex_t[:, spans[0][0]:spans[-1][1]],
                                    sc_t[:, spans[0][0]:spans[-1][1]],
                                    mybir.ActivationFunctionType.Exp,
                                    scale=0.125,
                                )
                            else:
                                for a, b_ in spans:
                                    nc.scalar.activation(
                                        ex_t[:, a:b_], sc_t[:, a:b_],
                                        mybir.ActivationFunctionType.Exp,
                                        scale=0.125,
                                    )
                            for _, off, nq, qo, do in subs:
                                if do is None:
                                    continue
                                # keep exp where q_local - k_local >= 0
                                nc.gpsimd.affine_select(
                                    out=ex_t[:, do:do + 128],
                                    in_=ex_t[:, do:do + 128],
                                    compare_op=mybir.AluOpType.is_ge,
                                    fill=0.0, base=0,
                                    pattern=[[1, 128]], channel_multiplier=-1,
                                )

                        def back():
                            if dve_exp:
                                # bit-read the int32 as f32, narrow to bf16;
                                # emitted back here so it queues AFTER the
                                # snapshot copies that gate the PE's psum WAR
                                a, b_ = spans[0][0], spans[-1][1]
                                nc.vector.tensor_copy(
                                    ex_t[:, a:b_],
                                    sx_t[:, a:b_].bitcast(F32),
                                )
                            for kc, off, nq, qo, _ in subs:
                                v1_t = v1_h[(pair, kc // QKC)]
                                vlo = 66 * (kc % QKC)
                                nc.tensor.matmul(
                                    po_t[:, qo:512],
                                    lhsT=v1_t[:, vlo:vlo + 65],
                                    rhs=ex_t[:, off:off + nq],
                                    start=(state["cc"] == 0),
                                    stop=(state["cc"] == total_cc - 1),
                                )
                                state["cc"] += 1
                            if has_snap:
                                si = state["si"]
                                state["si"] += 1
                                copy_eng.tensor_copy(
                                    otb_t[:, 512 * si:512 * (si + 1)],
                                    po_t[:, :])
                            if is_last:
                                # SBUF side must stay partition-first; the
                                # snapshot stride lives on the DRAM side
                                nc.sync.dma_start(
                                    out=ob_out[
                                        pair, :,
                                        base_idx:base_idx + nsnap_slot],
                                    in_=otb_t.rearrange("p (s c) -> p s c",
                                                        c=512),
                                )

                        return front, back

                    for gi, (subs, snap_w) in enumerate(groups):
                        out.append(mk(subs, snap_w is not None,
                                      gi == len(groups) - 1))
                return out

            # software pipeline: QK/exp of groups i+1..i+2 issue before the
            # AV of group i so the PE never head-of-line blocks on the
            # exp->mask latency or the snapshot copy WAR.
            pend = []
            depth = int(os.environ.get("KDEPTH", "2"))

            def pump(fb):
                front, back = fb
                front()
                pend.append(back)
                if len(pend) > depth:
                    pend.pop(0)()

            from collections import deque
            drip = deque()
            for c in range(len(couples)):
                drip.extend(prep_alloc(c))
            # first block's tiles up front; the rest stream behind compute
            for _ in range(3):
                drip.popleft()()
            for c, members in enumerate(couples):
                for pi, pair in enumerate(members):
                    for fb in build_pair_closures(pair):
                        pump(fb)
                        if drip:
                            drip.popleft()()
            while pend:
                pend.pop(0)()
    _split_excess_waits(nc)
    return nc


# ------------------------------------------------------------- host wrapper
_PROGRAM = None


def _get_program():
    global _PROGRAM
    if _PROGRAM is None:
        _PROGRAM = build_program()
    return _PROGRAM


_BF = ml_dtypes.bfloat16


def _marshal(qs, ks, vs):
    """[n_pairs, TS, 64] f32 triplet -> device input dict. Couple-packed
    bf16 Q^T (two pairs' d dims stacked on partitions), per-pair K^T with
    the other partition half zeroed (K=128 matmuls keep the PE warm), and
    the per-chunk strided [V/3 | 1] layout. Pure layout marshalling."""
    n_pairs = qs.shape[0]
    n_cpl = (n_pairs + 1) // 2
    QT = np.zeros((n_cpl, 128, TS), dtype=_BF)
    KT = np.zeros((n_pairs, 128, TS), dtype=_BF)
    for pair in range(n_pairs):
        lo = 64 * (pair % 2)
        QT[pair // 2, lo:lo + 64, :] = qs[pair].T.astype(_BF)
        KT[pair, lo:lo + 64, :] = ks[pair].T.astype(_BF)
    v = (vs.astype(np.float32) / 3.0).reshape(n_pairs, NCH, 128, 64)
    V1 = np.ones((n_pairs, 128, NCH, 66), dtype=_BF)
    V1[..., :64] = v.transpose(0, 2, 1, 3).astype(_BF)
    return {
        "QT": QT,
        "KT": KT,
        "V1": np.ascontiguousarray(V1.reshape(n_pairs, 128, NCH * 66)),
    }


def _shard_inputs(Q, K, V):
    """-> list of 8 dicts with permuted, marshalled per-core arrays."""
    in_maps = []
    for core in range(N_CORES):
        qs, ks, vs = [], [], []
        for pi in range(PAIRS_PER_CORE):
            flat = core * PAIRS_PER_CORE + pi
            b, h = flat // NH, flat % NH
            sm = _slot_map(h)
            for lst, src in ((qs, Q), (ks, K), (vs, V)):
                lst.append(
                    src[b, h].reshape(NBLK, BLK, D)[sm].reshape(TS, D)
                )
        in_maps.append(_marshal(np.stack(qs), np.stack(ks), np.stack(vs)))
    return in_maps


def _combine_outputs(results):
    out = np.zeros((B, NH, T, D), np.float32)
    for core in range(N_CORES):
        ob = results[core]["Ob"].astype(np.float32)  # [4, 65, NSNAP, 512]
        for pi in range(PAIRS_PER_CORE):
            flat = core * PAIRS_PER_CORE + pi
            b, h = flat // NH, flat % NH
            sm = _slot_map(h)
            _, snaps = _pair_schedule(pi % 2)
            blocks = np.zeros((NBLK, BLK, D), np.float32)
            for si, (slot, w) in enumerate(snaps):
                n = ob[pi, 0:64, si, :]          # (64, 512)
                den = ob[pi, 64, si, :]          # (512,)
                blocks[sm[slot]] += w * (n / den).T
            out[b, h] = blocks.reshape(T, D)
    return out


def kernel(Q, K, V):
    Q = np.asarray(Q, dtype=np.float32)
    K = np.asarray(K, dtype=np.float32)
    V = np.asarray(V, dtype=np.float32)
    nc = _get_program()
    in_maps = _shard_inputs(Q, K, V)
    res = run_bass_kernel_spmd(nc, in_maps, list(range(N_CORES)))
    return _combine_outputs(res.results)


if __name__ == "__main__":
    rng = np.random.default_rng(0)
    Q = rng.standard_normal((B, NH, T, D), dtype=np.float32)
    K = rng.standard_normal((B, NH, T, D), dtype=np.float32)
    V = rng.standard_normal((B, NH, T, D), dtype=np.float32)
    out = kernel(Q=Q, K=K, V=V)
    print("out", out.shape, out.dtype, float(np.abs(out).mean()))
